# revision 1
# baseline (speedup 1.0000x reference)
"""Trainium2 Bass kernel for: global-avg-pool -> 1x1conv -> GELU(exact) ->
1x1conv -> batched QR(Q only, LAPACK Householder sign convention) -> Q^T.

Full-input contract: kernel(**inputs) takes the unsharded inputs
(x[64,28,256,256], W1[28,28], b1[28], W2[168,28], b2[168]) and returns
[64,6,28] float32.  Internally: pure data-parallel over the batch dim on
8 NeuronCores (8 batches per core), no cross-core communication.
"""

import numpy as np

RANK = 6
C = 28          # channels
B = 64          # full batch
NCORES = 8
BPC = B // NCORES   # batches per core = 8
HW = 256 * 256      # pooled spatial size = 65536
RC = RANK * C       # 168

# erf(x/sqrt(2))/x = A0 + A1*x^2 + A2*x^4 + A3*x^6  (|x| <= ~0.25, f32-exact)
_A0 = float(np.sqrt(2.0 / np.pi))
_A1 = -_A0 / 6.0
_A2 = _A0 / 40.0
_A3 = -_A0 / 336.0


def build_nc(hw=HW, stage="full", iters=1):
    """Build the per-core Bass program (SPMD: same program on all cores).

    stage: "pool" (output raw channel sums in out[..224]), "gemm" (output
    M2 = y matrix), "full" (QR output).  iters>1 repeats the whole pipeline
    (for slope timing).
    """
    import concourse.bass as bass
    import concourse.bacc as bacc
    import concourse.mybir as mybir
    from concourse.tile import TileContext
    from contextlib import ExitStack

    dt = mybir.dt.float32
    AX = mybir.AxisListType
    ALU = mybir.AluOpType
    ACTF = mybir.ActivationFunctionType

    assert hw % 128 == 0
    fpp = hw // 128          # free elems per partition per plane

    nc = bacc.Bacc("TRN2", target_bir_lowering=False)
    x = nc.declare_dram_parameter("x", [BPC, C, hw], dt, isOutput=False)
    w1t = nc.declare_dram_parameter("w1t", [C, C], dt, isOutput=False)
    b1c = nc.declare_dram_parameter("b1c", [C, 1], dt, isOutput=False)
    w2t = nc.declare_dram_parameter("w2t", [C, RC], dt, isOutput=False)
    b2r = nc.declare_dram_parameter("b2r", [1, RC], dt, isOutput=False)
    einit = nc.declare_dram_parameter("einit", [BPC, RC], dt, isOutput=False)
    ones128 = nc.declare_dram_parameter("ones128", [128, 1], dt, isOutput=False)
    ones8 = nc.declare_dram_parameter("ones8", [1, BPC], dt, isOutput=False)
    out = nc.declare_dram_parameter("out", [BPC, RC], dt, isOutput=True)

    def rep_mid(ap, reps):
        # [P, F] -> [P, reps(broadcast), F]
        return bass.AP(ap.tensor, ap.offset, [ap.ap[0], [0, reps], ap.ap[1]])

    def rep_inner(ap, reps):
        # [P, F] -> [P, F, reps(broadcast)]
        return bass.AP(ap.tensor, ap.offset, [ap.ap[0], ap.ap[1], [0, reps]])

    with TileContext(nc) as tc, ExitStack() as ctx:
        constp = ctx.enter_context(tc.tile_pool(name="consts", bufs=1))
        xinp = ctx.enter_context(tc.tile_pool(name="xin", bufs=8))
        workp = ctx.enter_context(tc.tile_pool(name="work", bufs=1))
        psump = ctx.enter_context(tc.tile_pool(name="psum", bufs=1, space="PSUM"))

        # ---- constants to SBUF (SWDGE; overlaps with pooling DMAs) ----
        w1t_sb = constp.tile([C, C], dt, tag="w1t")
        nc.gpsimd.dma_start(w1t_sb[:], w1t[:])
        b1_sb = constp.tile([C, 1], dt, tag="b1")
        nc.gpsimd.dma_start(b1_sb[:], b1c[:])
        w2t_sb = constp.tile([C, RC], dt, tag="w2t")
        nc.gpsimd.dma_start(w2t_sb[:], w2t[:])
        b2_sb = constp.tile([1, RC], dt, tag="b2")
        nc.gpsimd.dma_start(b2_sb[:], b2r[:])
        ones128_sb = constp.tile([128, 1], dt, tag="o128")
        nc.gpsimd.dma_start(ones128_sb[:], ones128[:])
        ones8_sb = constp.tile([1, BPC], dt, tag="o8")
        nc.gpsimd.dma_start(ones8_sb[:], ones8[:])

        for _it in range(iters):
            # ---- pooling: per-channel DMA of 8 planes, free-dim reduce ----
            # S[p, c*8+b] = sum_f x[b, c, p*fpp + f]
            # x DMAs issue from the ACT engine (HWDGE). Before each slot-reuse
            # DMA, a tiny ACT "carrier" op reads the freeing reduce's output so
            # the ACT sequencer observes the DVE tick and the DMA keeps a single
            # sync wait (its lane WAW) -- avoids extra EventSemaphore stalls.
            S = workp.tile([128, C * BPC], dt, tag="S")
            dummy = workp.tile([1, C], dt, tag="dummy")
            nbufs = 8
            for c in range(C):
                if c >= nbufs:
                    nc.scalar.activation(
                        dummy[0:1, c - nbufs:c - nbufs + 1],
                        S[0:1, (c - nbufs) * BPC:(c - nbufs) * BPC + 1],
                        ACTF.Copy,
                    )
                t = xinp.tile([128, BPC * fpp], dt, tag="xin")
                src = x[:, c, :].rearrange("b (p f) -> p b f", p=128)
                nc.scalar.dma_start(t[:], src)
                nc.vector.reduce_sum(
                    S[:, c * BPC:(c + 1) * BPC],
                    t[:].rearrange("p (b f) -> p b f", b=BPC),
                    axis=AX.X,
                )

            # 1x1 PE carrier matmuls: absorb one operand's sem into PE's
            # observed clock before each real matmul.
            psum_dmy = psump.tile([1, 1], dt, tag="pdmy")

            def pe_carrier(src):
                nc.tensor.matmul(psum_dmy[:], src, src, start=True, stop=True)

            # ---- stage 2: p_t[c, b] = sum_p S[p, c*8+b], sums on partitions:
            # one matmul per batch, lhsT = S[:, b::8] (strided), rhs = ones128.
            psum_pt = psump.tile([C, BPC], dt, tag="ppt")
            pe_carrier(S[0:1, (C - 1) * BPC:(C - 1) * BPC + 1])
            S_r = S[:].rearrange("p (c b) -> p b c", b=BPC)
            for b in range(BPC):
                nc.tensor.matmul(
                    psum_pt[:, b:b + 1], S_r[:, b, :], ones128_sb[:],
                    start=True, stop=True,
                )
            p_t = workp.tile([C, BPC], dt, tag="pt")
            nc.scalar.activation(p_t[:], psum_pt[:], ACTF.Copy)

            if stage == "pool":
                dst = out[:].rearrange("b f -> (b f)")[0:C * BPC]
                nc.scalar.dma_start(dst, p_t[:])
            else:
                # ---- GEMM1 (1/HW folded in w1t) + bias + exact gelu poly ----
                psum_h = psump.tile([C, BPC], dt, tag="ph")
                pe_carrier(p_t[0:1, 0:1])
                nc.tensor.matmul(psum_h[:], w1t_sb[:], p_t[:], start=True, stop=True)
                xh = workp.tile([C, BPC], dt, tag="xh")
                nc.scalar.activation(xh[:], psum_h[:], ACTF.Identity,
                                     bias=b1_sb[:], scale=1.0)
                tsq = workp.tile([C, BPC], dt, tag="tsq")
                nc.scalar.activation(tsq[:], xh[:], ACTF.Square)
                u = workp.tile([C, BPC], dt, tag="u")
                nc.vector.tensor_scalar(u[:], tsq[:], _A3, _A2, ALU.mult, ALU.add)
                nc.vector.tensor_tensor(u[:], u[:], tsq[:], ALU.mult)
                nc.vector.tensor_scalar(u[:], u[:], _A1, None, ALU.add)
                nc.vector.tensor_tensor(u[:], u[:], tsq[:], ALU.mult)
                nc.vector.tensor_scalar(u[:], u[:], _A0, None, ALU.add)
                nc.vector.tensor_tensor(u[:], u[:], xh[:], ALU.mult)  # erf(x/rt2)
                nc.vector.tensor_scalar(u[:], u[:], 1.0, None, ALU.add)
                ht = workp.tile([C, BPC], dt, tag="ht")
                nc.vector.tensor_tensor(ht[:], xh[:], u[:], ALU.mult)  # 2*gelu

                # ---- GEMM2 (0.5 folded in w2t) + bias via ones outer product ----
                psum_y = psump.tile([BPC, RC], dt, tag="py")
                pe_carrier(ht[0:1, 0:1])
                nc.tensor.matmul(psum_y[:], ht[:], w2t_sb[:], start=True, stop=False)
                pe_carrier(b2_sb[0:1, 0:1])
                nc.tensor.matmul(psum_y[:], ones8_sb[:], b2_sb[:],
                                 start=False, stop=True)
                M2 = workp.tile([BPC, RC], dt, tag="M2")
                nc.vector.tensor_copy(M2[:], psum_y[:])

                if stage == "gemm":
                    nc.scalar.activation(dummy[0:1, 0:1], M2[0:1, 0:1], ACTF.Copy)
                    nc.scalar.dma_start(out[:], M2[:])
                else:
                    # ---- batched Householder QR (LAPACK sign convention) ----
                    # row b of M2 = A_b^T: group j = column j of A_b (28-vec)
                    V2 = workp.tile([BPC, RC], dt, tag="V2")
                    nc.vector.memset(V2[:], 0.0)
                    Wt = workp.tile([BPC, RC], dt, tag="Wt")
                    Qw = workp.tile([BPC, RC], dt, tag="Qw")
                    nc.gpsimd.dma_start(Qw[:], einit[:])
                    prod = workp.tile([BPC, RC], dt, tag="prod")
                    upd = workp.tile([BPC, RC], dt, tag="upd")
                    dots = workp.tile([BPC, RANK], dt, tag="dots")
                    nrm2 = workp.tile([BPC, 1], dt, tag="nrm2")
                    svec = workp.tile([BPC, 1], dt, tag="svec")
                    nsg = workp.tile([BPC, 1], dt, tag="nsg")
                    beta = workp.tile([BPC, 1], dt, tag="beta")
                    dvec = workp.tile([BPC, 1], dt, tag="dvec")
                    cvec = workp.tile([BPC, 1], dt, tag="cvec")
                    scr = workp.tile([BPC, C], dt, tag="scr")

                    M2v = M2[:].rearrange("b (r c) -> b r c", r=RANK)
                    prodv = prod[:].rearrange("b (r c) -> b r c", r=RANK)
                    updv = upd[:].rearrange("b (r c) -> b r c", r=RANK)


                    def apply_reflector(k, target, targetv):
                        # target -= (c*v) * (v . target_rows) for every rank-row
                        nc.vector.tensor_tensor(
                            prodv, targetv,
                            rep_mid(V2[:, k * C:(k + 1) * C], RANK), ALU.mult
                        )
                        nc.vector.reduce_sum(dots[:], prodv, axis=AX.X)
                        nc.vector.tensor_tensor(
                            updv,
                            rep_mid(Wt[:, k * C:(k + 1) * C], RANK),
                            rep_inner(dots[:], C),
                            ALU.mult,
                        )
                        nc.vector.tensor_tensor(target[:], target[:], upd[:],
                                                ALU.subtract)

                    nop = 99
                    if stage.startswith("qr_op"):
                        nop = int(stage[5:])
                    nsteps = 0 if stage == "qr_init" else (
                        1 if (stage in ("qr_scal", "qr_refl")
                              or stage.startswith("qr_op")) else RANK)
                    for k in range(nsteps):
                        col = k * C + k
                        gend = (k + 1) * C
                        xk = M2[:, col:gend]
                        nc.vector.tensor_tensor(scr[:, :C - k], xk, xk, ALU.mult)
                        nc.vector.reduce_sum(nrm2[:], scr[:, :C - k], axis=AX.X)
                        if nop < 2:
                            break
                        nc.scalar.activation(svec[:], nrm2[:], ACTF.Sqrt)
                        if nop < 3:
                            break
                        nc.scalar.activation(nsg[:], M2[:, col:col + 1], ACTF.Sign,
                                             scale=-1.0)
                        if nop < 4:
                            break
                        nc.vector.tensor_scalar(beta[:], svec[:], nsg[:], None,
                                                ALU.mult)
                        if nop < 5:
                            break
                        # v = x, v[0] = alpha - beta
                        nc.vector.tensor_copy(V2[:, col:gend], xk)
                        nc.vector.tensor_scalar(
                            V2[:, col:col + 1], M2[:, col:col + 1], beta[:], None,
                            ALU.subtract,
                        )
                        if nop < 6:
                            break
                        # d = (beta - alpha) * beta ; c = 1/d
                        nc.vector.tensor_scalar(
                            dvec[:], beta[:], M2[:, col:col + 1], beta[:],
                            ALU.subtract, ALU.mult,
                        )
                        if nop < 7:
                            break
                        nc.vector.reciprocal(cvec[:], dvec[:])
                        if nop < 8:
                            break
                        # w = c * v
                        nc.vector.tensor_scalar(
                            Wt[:, k * C:gend], V2[:, k * C:gend], cvec[:], None,
                            ALU.mult,
                        )
                        if stage != "qr_scal" and nop >= 99:
                            apply_reflector(k, M2, M2v)

                    if stage == "full":
                        Qwv = Qw[:].rearrange("b (r c) -> b r c", r=RANK)
                        for k in reversed(range(RANK)):
                            apply_reflector(k, Qw, Qwv)
                        probe = Qw
                    elif stage == "qr_init":
                        probe = Qw
                    elif stage == "qr_scal":
                        probe = Wt
                    elif stage.startswith("qr_op"):
                        probe = V2
                    else:   # qr_refl
                        probe = M2

                    # carrier: ACT observes the final DVE write of probe, so the
                    # out-DMA needs only one sync wait
                    nc.scalar.activation(dummy[0:1, 0:1], probe[0:1, 0:1], ACTF.Copy)
                    nc.scalar.dma_start(out[:], probe[:])

    nc.compile()   # bacc passes incl. generate_event_semaphores (1-wait limit)
    return nc


def host_inputs(x_shard, W1, b1, W2, b2, hw=HW):
    """Per-core input map. x_shard: [BPC, C, hw] f32."""
    w1t = (W1.T / np.float32(hw)).astype(np.float32)          # [28, 28]
    w2t = (0.5 * W2.T).astype(np.float32)                     # [28, 168]
    e = np.zeros((BPC, RC), dtype=np.float32)
    for j in range(RANK):
        e[:, j * C + j] = 1.0
    return {
        "x": np.ascontiguousarray(x_shard.reshape(BPC, C, hw)),
        "w1t": np.ascontiguousarray(w1t),
        "b1c": np.ascontiguousarray(b1.reshape(C, 1).astype(np.float32)),
        "w2t": np.ascontiguousarray(w2t),
        "b2r": np.ascontiguousarray(b2.reshape(1, RC).astype(np.float32)),
        "einit": e,
        "ones128": np.ones((128, 1), dtype=np.float32),
        "ones8": np.ones((1, BPC), dtype=np.float32),
    }


_CACHED_NC = None


def kernel(x, W1, b1, W2, b2, trace=False):
    from concourse.bass_utils import run_bass_kernel_spmd

    global _CACHED_NC
    if _CACHED_NC is None:
        _CACHED_NC = build_nc()
    nc = _CACHED_NC

    x = np.asarray(x, dtype=np.float32).reshape(B, C, HW)
    in_maps = []
    for i in range(NCORES):
        in_maps.append(
            host_inputs(x[i * BPC:(i + 1) * BPC], np.asarray(W1), np.asarray(b1),
                        np.asarray(W2), np.asarray(b2))
        )
    res = run_bass_kernel_spmd(nc, in_maps, list(range(NCORES)), trace=trace)
    outs = [np.asarray(res.results[i]["out"]).reshape(BPC, RANK, C)
            for i in range(NCORES)]
    full = np.concatenate(outs, axis=0)
    if trace:
        return full, res
    return full



# revision 8
# speedup vs baseline: 1.0896x; 1.0896x over previous
"""Trainium2 Bass kernel for: global-avg-pool -> 1x1conv -> GELU(exact) ->
1x1conv -> batched QR(Q only, LAPACK Householder sign convention) -> Q^T.

Full-input contract: kernel(**inputs) takes the unsharded inputs
(x[64,28,256,256], W1[28,28], b1[28], W2[168,28], b2[168]) and returns
[64,6,28] float32.  Internally: pure data-parallel over the batch dim on
8 NeuronCores (8 batches per core), no cross-core communication.

v2 pooling front-end: the hw-dim partial sums run on the PE (float32r
matmuls with a block-indicator stationary operand, PSUM accumulation over
512-wide chunks), leaving the DVE almost idle; x is DMA'd in 3 channel-
group tiles per batch (16/8/4 channels -> 32/16/8KB contiguous lines).
"""

import numpy as np

RANK = 6
C = 28          # channels
B = 64          # full batch
NCORES = 8
BPC = B // NCORES   # batches per core = 8
HW = 256 * 256      # pooled spatial size = 65536
RC = RANK * C       # 168

# channel groups per batch: (start, k); k*512 f32 contiguous per partition
CH_GROUPS = [(0, 16), (16, 8), (24, 4)]

# erf(x/sqrt(2))/x = A0 + A1*x^2 + A2*x^4 + A3*x^6  (|x| <= ~0.25, f32-exact)
_A0 = float(np.sqrt(2.0 / np.pi))
_A1 = -_A0 / 6.0
_A2 = _A0 / 40.0
_A3 = -_A0 / 336.0


def build_nc(hw=HW, stage="full", iters=1):
    """Build the per-core Bass program (SPMD: same program on all cores)."""
    import concourse.bass as bass
    import concourse.bacc as bacc
    import concourse.mybir as mybir
    from concourse.tile import TileContext
    from contextlib import ExitStack

    dt = mybir.dt.float32
    dtr = mybir.dt.float32r
    AX = mybir.AxisListType
    ALU = mybir.AluOpType
    ACTF = mybir.ActivationFunctionType

    assert hw % 128 == 0
    fpp = hw // 128          # free elems per partition for a 1-channel tile

    nc = bacc.Bacc("TRN2", target_bir_lowering=False)
    x = nc.declare_dram_parameter("x", [BPC, C, hw], dtr, isOutput=False)
    w1t = nc.declare_dram_parameter("w1t", [C, C], dt, isOutput=False)
    b1c = nc.declare_dram_parameter("b1c", [C, 1], dt, isOutput=False)
    w2t = nc.declare_dram_parameter("w2t", [C, RC], dt, isOutput=False)
    b2r = nc.declare_dram_parameter("b2r", [1, RC], dt, isOutput=False)
    einit = nc.declare_dram_parameter("einit", [BPC, RC], dt, isOutput=False)
    inds = nc.declare_dram_parameter("inds", [128, 28], dtr, isOutput=False)
    ones8 = nc.declare_dram_parameter("ones8", [1, BPC], dt, isOutput=False)
    out = nc.declare_dram_parameter("out", [BPC, RC], dt, isOutput=True)

    def rep_mid(ap, reps):
        # [P, F] -> [P, reps(broadcast), F]
        return bass.AP(ap.tensor, ap.offset, [ap.ap[0], [0, reps], ap.ap[1]])

    def rep_inner(ap, reps):
        # [P, F] -> [P, F, reps(broadcast)]
        return bass.AP(ap.tensor, ap.offset, [ap.ap[0], ap.ap[1], [0, reps]])

    with TileContext(nc) as tc, ExitStack() as ctx:
        constp = ctx.enter_context(tc.tile_pool(name="consts", bufs=1))
        xinp = ctx.enter_context(tc.tile_pool(name="xin", bufs=2))
        workp = ctx.enter_context(tc.tile_pool(name="work", bufs=1))
        pbank = ctx.enter_context(tc.tile_pool(name="pbank", bufs=1,
                                               space="PSUM"))
        psump = ctx.enter_context(tc.tile_pool(name="psum", bufs=1,
                                               space="PSUM"))

        # ---- constants to SBUF (SWDGE; overlaps with pooling DMAs) ----
        # W1^T row-blocks per channel group (engine base-partition rule)
        w1g = {}
        for (cs, k) in CH_GROUPS:
            w1g_t = constp.tile([k, C], dt, tag=f"w1g{k}")
            nc.gpsimd.dma_start(w1g_t[:], w1t[cs:cs + k, :])
            w1g[k] = w1g_t[:]
        b1_sb = constp.tile([C, 1], dt, tag="b1")
        nc.gpsimd.dma_start(b1_sb[:], b1c[:])
        w2t_sb = constp.tile([C, RC], dt, tag="w2t")
        nc.gpsimd.dma_start(w2t_sb[:], w2t[:])
        b2_sb = constp.tile([1, RC], dt, tag="b2")
        nc.gpsimd.dma_start(b2_sb[:], b2r[:])
        inds_sb = constp.tile([128, 28], dtr, tag="inds")
        nc.gpsimd.dma_start(inds_sb[:], inds[:])
        ones8_sb = constp.tile([1, BPC], dt, tag="o8")
        nc.gpsimd.dma_start(ones8_sb[:], ones8[:])
        # indicator column blocks within inds_sb: [:, 0:16]=ind16,
        # [:, 16:24]=ind8, [:, 24:28]=ind4  (see host_inputs)
        IND_OFF = {16: 0, 8: 16, 4: 24}

        for _it in range(iters):
            # ---- pooling stage A on PE: per-(batch, group) PSUM tiles ----
            # (engine APs must start at partition 0/32/64/96 -> one tile per
            # channel group, each starting at partition 0; channel sums land
            # in per-group SBUF tiles pg[k] with batch on the free dim)
            pg = {}
            for (_cs, k) in CH_GROUPS:
                pg_t = workp.tile([k, BPC], dt, tag=f"pg{k}")
                pg[k] = pg_t
            for b in range(BPC):
                for (cs, k) in CH_GROUPS:
                    t = xinp.tile([128, k * 512], dtr, tag=f"xin{k}")
                    src = x[b, cs:cs + k, :].rearrange(
                        "c (q f) -> (c q) f", f=k * 512)
                    nc.scalar.dma_start(t[:], src)
                    ind = inds_sb[:, IND_OFF[k]:IND_OFF[k] + k]
                    pb = pbank.tile([k, 512], dt, tag=f"pb{k}_{b % 2}")
                    for j in range(k):
                        nc.tensor.matmul(
                            pb[:],
                            ind,
                            t[:, j * 512:(j + 1) * 512],
                            start=(j == 0), stop=(j == k - 1),
                        )
                    # ---- stage B: free-dim reduce -> channel sums
                    nc.vector.reduce_sum(pg[k][:, b:b + 1], pb[:], axis=AX.X)

            if stage == "pool":
                dummy = workp.tile([1, C], dt, tag="dummy")
                nc.scalar.activation(dummy[0:1, 0:1], pg[4][0:1, 0:1],
                                     ACTF.Copy)
                flat = out[:].rearrange("b f -> (b f)")
                off = 0
                for (_cs, k) in CH_GROUPS:
                    nc.scalar.dma_start(flat[off:off + k * BPC], pg[k][:])
                    off += k * BPC
            else:
                dummy = workp.tile([1, C], dt, tag="dummy")
                psum_h = psump.tile([C, BPC], dt, tag="ph")

                def pe_carrier(src):
                    # absorb one operand's sem into PE's observed clock;
                    # psum_h[0:1,0:1] is dead/reset at every carrier point
                    nc.tensor.matmul(psum_h[0:1, 0:1], src, src, start=True,
                                     stop=True)

                # ---- GEMM1 (1/HW folded in w1t) + bias + exact gelu poly ----
                # contract over c in 3 row-blocks (one per channel group)
                pe_carrier(pg[4][0:1, 0:1])
                ngr = len(CH_GROUPS)
                for gi, (cs, k) in enumerate(CH_GROUPS):
                    nc.tensor.matmul(psum_h[:], w1g[k], pg[k][:],
                                     start=(gi == 0), stop=(gi == ngr - 1))
                xh = workp.tile([C, BPC], dt, tag="xh")
                nc.scalar.activation(xh[:], psum_h[:], ACTF.Identity,
                                     bias=b1_sb[:], scale=1.0)
                tsq = workp.tile([C, BPC], dt, tag="tsq")
                nc.scalar.activation(tsq[:], xh[:], ACTF.Square)
                u = workp.tile([C, BPC], dt, tag="u")
                nc.vector.tensor_scalar(u[:], tsq[:], _A3, _A2, ALU.mult,
                                        ALU.add)
                nc.vector.tensor_tensor(u[:], u[:], tsq[:], ALU.mult)
                nc.vector.tensor_scalar(u[:], u[:], _A1, None, ALU.add)
                nc.vector.tensor_tensor(u[:], u[:], tsq[:], ALU.mult)
                nc.vector.tensor_scalar(u[:], u[:], _A0, None, ALU.add)
                nc.vector.tensor_tensor(u[:], u[:], xh[:], ALU.mult)
                nc.vector.tensor_scalar(u[:], u[:], 1.0, None, ALU.add)
                ht = workp.tile([C, BPC], dt, tag="ht")
                nc.vector.tensor_tensor(ht[:], xh[:], u[:], ALU.mult)  # 2*gelu

                # ---- GEMM2 (0.5 folded in w2t) + bias via ones outer ----
                psum_y = psump.tile([BPC, RC], dt, tag="py")
                pe_carrier(ht[0:1, 0:1])
                nc.tensor.matmul(psum_y[:], ht[:], w2t_sb[:], start=True,
                                 stop=False)
                pe_carrier(b2_sb[0:1, 0:1])
                nc.tensor.matmul(psum_y[:], ones8_sb[:], b2_sb[:],
                                 start=False, stop=True)
                M2 = workp.tile([BPC, RC], dt, tag="M2")
                nc.vector.tensor_copy(M2[:], psum_y[:])

                if stage == "gemm":
                    nc.scalar.activation(dummy[0:1, 0:1], M2[0:1, 0:1],
                                         ACTF.Copy)
                    nc.scalar.dma_start(out[:], M2[:])
                else:
                    # ---- batched Householder QR (LAPACK sign convention) ----
                    V2 = workp.tile([BPC, RC], dt, tag="V2")
                    nc.vector.memset(V2[:], 0.0)
                    Wt = workp.tile([BPC, RC], dt, tag="Wt")
                    Qw = workp.tile([BPC, RC], dt, tag="Qw")
                    nc.gpsimd.dma_start(Qw[:], einit[:])
                    prod = workp.tile([BPC, RC], dt, tag="prod")
                    upd = workp.tile([BPC, RC], dt, tag="upd")
                    dots = workp.tile([BPC, RANK], dt, tag="dots")
                    nrm2 = workp.tile([BPC, 1], dt, tag="nrm2")
                    svec = workp.tile([BPC, 1], dt, tag="svec")
                    nsg = workp.tile([BPC, 1], dt, tag="nsg")
                    beta = workp.tile([BPC, 1], dt, tag="beta")
                    dvec = workp.tile([BPC, 1], dt, tag="dvec")
                    cvec = workp.tile([BPC, 1], dt, tag="cvec")
                    scr = workp.tile([BPC, C], dt, tag="scr")

                    M2v = M2[:].rearrange("b (r c) -> b r c", r=RANK)
                    prodv = prod[:].rearrange("b (r c) -> b r c", r=RANK)
                    updv = upd[:].rearrange("b (r c) -> b r c", r=RANK)

                    def apply_reflector(k, target, targetv):
                        nc.vector.tensor_tensor(
                            prodv, targetv,
                            rep_mid(V2[:, k * C:(k + 1) * C], RANK), ALU.mult
                        )
                        nc.vector.reduce_sum(dots[:], prodv, axis=AX.X)
                        nc.vector.tensor_tensor(
                            updv,
                            rep_mid(Wt[:, k * C:(k + 1) * C], RANK),
                            rep_inner(dots[:], C),
                            ALU.mult,
                        )
                        nc.vector.tensor_tensor(target[:], target[:], upd[:],
                                                ALU.subtract)

                    for k in range(RANK):
                        col = k * C + k
                        gend = (k + 1) * C
                        xk = M2[:, col:gend]
                        nc.vector.tensor_tensor(scr[:, :C - k], xk, xk,
                                                ALU.mult)
                        nc.vector.reduce_sum(nrm2[:], scr[:, :C - k],
                                             axis=AX.X)
                        nc.scalar.activation(svec[:], nrm2[:], ACTF.Sqrt)
                        nc.scalar.activation(nsg[:], M2[:, col:col + 1],
                                             ACTF.Sign, scale=-1.0)
                        nc.vector.tensor_scalar(beta[:], svec[:], nsg[:],
                                                None, ALU.mult)
                        # v = x, v[0] = alpha - beta
                        nc.vector.tensor_copy(V2[:, col:gend], xk)
                        nc.vector.tensor_scalar(
                            V2[:, col:col + 1], M2[:, col:col + 1], beta[:],
                            None, ALU.subtract,
                        )
                        # d = (beta - alpha) * beta ; c = 1/d
                        nc.vector.tensor_scalar(
                            dvec[:], beta[:], M2[:, col:col + 1], beta[:],
                            ALU.subtract, ALU.mult,
                        )
                        nc.vector.reciprocal(cvec[:], dvec[:])
                        # w = c * v
                        nc.vector.tensor_scalar(
                            Wt[:, k * C:gend], V2[:, k * C:gend], cvec[:],
                            None, ALU.mult,
                        )
                        apply_reflector(k, M2, M2v)

                    Qwv = Qw[:].rearrange("b (r c) -> b r c", r=RANK)
                    for k in reversed(range(RANK)):
                        apply_reflector(k, Qw, Qwv)

                    # carrier: ACT observes the final DVE write of Qw, so the
                    # out-DMA needs only one sync wait
                    nc.scalar.activation(dummy[0:1, 0:1], Qw[0:1, 0:1],
                                         ACTF.Copy)
                    nc.scalar.dma_start(out[:], Qw[:])

    nc.compile()
    return nc


def host_inputs(x_shard, W1, b1, W2, b2, hw=HW):
    """Per-core input map. x_shard: [BPC, C, hw] f32."""
    w1t = (W1.T / np.float32(hw)).astype(np.float32)          # [28, 28]
    w2t = (0.5 * W2.T).astype(np.float32)                     # [28, 168]
    e = np.zeros((BPC, RC), dtype=np.float32)
    for j in range(RANK):
        e[:, j * C + j] = 1.0
    # block indicators: column blocks [ind16 | ind8 | ind4]
    inds = np.zeros((128, 28), dtype=np.float32)
    for k, off in ((16, 0), (8, 16), (4, 24)):
        q = 128 // k
        for i in range(k):
            inds[i * q:(i + 1) * q, off + i] = 1.0
    return {
        "x": np.ascontiguousarray(x_shard.reshape(BPC, C, hw)),
        "w1t": np.ascontiguousarray(w1t),
        "b1c": np.ascontiguousarray(b1.reshape(C, 1).astype(np.float32)),
        "w2t": np.ascontiguousarray(w2t),
        "b2r": np.ascontiguousarray(b2.reshape(1, RC).astype(np.float32)),
        "einit": e,
        "inds": inds,
        "ones8": np.ones((1, BPC), dtype=np.float32),
    }


_CACHED_NC = None


def kernel(x, W1, b1, W2, b2, trace=False):
    from concourse.bass_utils import run_bass_kernel_spmd

    global _CACHED_NC
    if _CACHED_NC is None:
        _CACHED_NC = build_nc()
    nc = _CACHED_NC

    x = np.asarray(x, dtype=np.float32).reshape(B, C, HW)
    in_maps = []
    for i in range(NCORES):
        in_maps.append(
            host_inputs(x[i * BPC:(i + 1) * BPC], np.asarray(W1),
                        np.asarray(b1), np.asarray(W2), np.asarray(b2))
        )
    res = run_bass_kernel_spmd(nc, in_maps, list(range(NCORES)), trace=trace)
    outs = [np.asarray(res.results[i]["out"]).reshape(BPC, RANK, C)
            for i in range(NCORES)]
    full = np.concatenate(outs, axis=0)
    if trace:
        return full, res
    return full
